# revision 1
# baseline (speedup 1.0000x reference)
"""Trainium2 Bass kernel for nn_Attention (B=4, L=1024, D=768, H=12, DH=64).

Reference per (batch b, head n):
    K = k_n @ x_b^T; Q = q_n @ x_b^T        [D, L]
    scores = Q^T K                          [L, L]
    S = softmax(scores, -1)
    V = v_n @ x_b^T                         [DH, L]
    out[b, l, n*DH+e] = sum_m S[l, m] V[e, m]

Sharding: 48 independent (b, n) units over 8 cores.  Core c owns the
batch PAIR bp = c//4 (batches 2bp, 2bp+1) and the head TRIPLE t = c%4
(heads 3t..3t+3): A = q^T k is computed once per head and reused for
both batches.  For heads 1 and 2 the two cores that share a head
triple (c and c+4) each compute one half of A (the host hands each
core its own 384-column slice of k) and exchange halves with a 2-rank
AllGather that overlaps head-0 compute.  Head 0's A is computed fully
locally since it sits on the critical path.

Device math per (head, batch) with all-bf16 matmuls / f32 PSUM:
    A  = q_n^T k_n                [D, D]
    WT = A^T x_b^T                [D, L]
    sT(mj) = xtT-block^T @ WT     [128m, L]  (scores transposed)
    pT(mj) = exp(sT(mj))          bf16       (no max subtraction:
                                              logits are O(1) here)
    R^T[l-block, 0:64] + sums[l] via matmul with vt_aug (V^T plus a
    ones column -> column 64 accumulates sum_m exp) accumulated over mj
    out_block = R^T * (1/sums)    per-partition tensor_scalar
Output per core: out_r [2, L, 192]; host writes out_r[bi] straight
into out[2bp+bi, :, 192t:192t+192].
"""

from contextlib import ExitStack

import ml_dtypes
import numpy as np

import concourse.tile as tile
from concourse import bacc, mybir
from concourse.bass import ts, ds
from concourse.bass_utils import run_bass_kernel_spmd

# If BASS_TRACE is set in an environment that lacks antenv.axon_hooks,
# run_bass_kernel_spmd's trace path would fail on import; register a
# fallback holder so tracing degrades gracefully instead.
try:
    import antenv.axon_hooks  # noqa: F401
except Exception:  # pragma: no cover
    import sys
    import types

    import antenv

    _m = types.ModuleType("antenv.axon_hooks")
    _m._hook = None
    _m.set_axon_ntff_profile_hook = lambda h: setattr(_m, "_hook", h)
    _m.get_axon_ntff_profile_hook = lambda: _m._hook
    sys.modules["antenv.axon_hooks"] = _m
    antenv.axon_hooks = _m

B, L, D, H = 4, 1024, 768, 12
DH = D // H          # 64
HPC = 3              # heads per core
BPC = 2              # batches per core
N_CORES = 8
DC = D // 128        # 6 chunks of the contraction/feature dim
LB = L // 128        # 8 l-blocks / m-blocks
DHA = DH + 1         # 65: head slice width in vt_aug (ones column at 64)
DHALF = D // 2       # 384: A-half width for the pairwise exchange
F32 = mybir.dt.float32
BF16 = mybir.dt.bfloat16
PAIR_GROUPS = [[0, 4], [1, 5], [2, 6], [3, 7]]

_COMPILED = None


def _build():
    nc = bacc.Bacc(
        "TRN2",
        target_bir_lowering=False,
        debug=False,
        enable_asserts=False,
        num_devices=N_CORES,
    )
    xT_ext = nc.dram_tensor("xT", [BPC, D, L], BF16, kind="ExternalInput").ap()
    q3_ext = nc.dram_tensor("q3", [HPC, D, D], BF16, kind="ExternalInput").ap()
    k0_ext = nc.dram_tensor("k0", [D, D], BF16, kind="ExternalInput").ap()
    kh_ext = nc.dram_tensor("kh", [2, D, DHALF], BF16, kind="ExternalInput").ap()
    vT3_ext = nc.dram_tensor("vT3", [D, HPC * DH], BF16, kind="ExternalInput").ap()
    out_ext = nc.dram_tensor(
        "out_r", [BPC, L, HPC * DH], F32, kind="ExternalOutput"
    ).ap()

    with tile.TileContext(nc) as tc, ExitStack() as ctx:
        xt_pool = ctx.enter_context(tc.tile_pool(name="xt", bufs=1))
        vt3_pool = ctx.enter_context(tc.tile_pool(name="vt3", bufs=1))
        vt_pool = ctx.enter_context(tc.tile_pool(name="vt", bufs=1))
        qk_pool = ctx.enter_context(tc.tile_pool(name="qk", bufs=1))
        a_pool = ctx.enter_context(tc.tile_pool(name="a", bufs=1))
        wt_pool = ctx.enter_context(tc.tile_pool(name="wt", bufs=1))
        pt_pool = ctx.enter_context(tc.tile_pool(name="pt", bufs=2))
        soft_pool = ctx.enter_context(tc.tile_pool(name="soft", bufs=2))
        out_pool = ctx.enter_context(tc.tile_pool(name="outp", bufs=1))
        dram_pool = ctx.enter_context(tc.tile_pool(name="dram", bufs=1, space="DRAM"))
        ps_p = ctx.enter_context(tc.tile_pool(name="ps_p", bufs=2, space="PSUM"))
        ps_s = ctx.enter_context(tc.tile_pool(name="ps_s", bufs=2, space="PSUM"))
        ps_r = ctx.enter_context(tc.tile_pool(name="ps_r", bufs=2, space="PSUM"))

        # ---------- loads: one big 3D-AP DMA per tensor ----------
        # tile[p, c, :] = dram row c*128+p; fewest dma_starts so the early
        # stream is bandwidth- not issue-rate-limited.
        def load3d(pool, tag, dram2d, width):
            t = pool.tile([128, DC, width], BF16, tag=tag)
            nc.sync.dma_start(
                t[:], dram2d.rearrange("(c p) w -> p c w", p=128)
            )
            return t

        # xt[b0] + vt3 (1.8 MB) unblock VT(b0); q0/k0 stream during it.
        xt = [None, None]
        xt[0] = load3d(xt_pool, "xt0", xT_ext[0], L)
        vt3 = load3d(vt3_pool, "vt3", vT3_ext[:], HPC * DH)
        q_all = [None, None, None]
        q_all[0] = load3d(qk_pool, "q0", q3_ext[0], D)
        k0_sb = load3d(qk_pool, "k0", k0_ext[:], D)
        xt[1] = load3d(xt_pool, "xt1", xT_ext[1], L)
        kh_sb = [None, None]
        for h in (1, 2):
            q_all[h] = load3d(qk_pool, f"q{h}", q3_ext[h], D)
            kh_sb[h - 1] = load3d(qk_pool, f"kh{h}", kh_ext[h - 1], DHALF)

        # a tiles per head: lo = A[:, 0:384], hi = A[:, 384:768]
        a_lo = [[], [], []]
        a_hi = [[], [], []]

        # ---------- VT_aug projection per batch ----------
        vt = [None, None]

        def build_vt(bi):
            tiles = []
            for j in range(LB):
                pool = ps_p if j % 2 == 0 else ps_r
                p = pool.tile([128, 512], F32, tag=pool.name)
                for i in range(DC):
                    nc.tensor.matmul(
                        p[:, : HPC * DH],
                        xt[bi][:, i, ts(j, 128)],
                        vt3[:, i, :],
                        start=(i == 0),
                        stop=(i == DC - 1),
                    )
                t = vt_pool.tile([128, HPC * DHA], BF16, tag=f"vt{bi}_{j}")
                nc.gpsimd.memset(t[:], 1.0)
                t3 = t[:].rearrange("p (h c) -> p h c", h=HPC)
                p3 = p[:, : HPC * DH].rearrange("p (h c) -> p h c", h=HPC)
                nc.vector.tensor_copy(t3[:, :, :DH], p3[:])
                tiles.append(t)
            vt[bi] = tiles

        build_vt(0)  # first PE work: needs only xt[b0] + vt3

        # ---- head 0: full A locally ----
        for i in range(DC):
            tl = a_pool.tile([128, DHALF], BF16, tag=f"alo0_{i}")
            th = a_pool.tile([128, DHALF], BF16, tag=f"ahi0_{i}")
            for n, t in ((0, tl), (1, th)):
                pool = ps_p if n == 0 else ps_r
                p = pool.tile([128, 512], F32, tag=pool.name)
                for j in range(DC):
                    nc.tensor.matmul(
                        p[:, :DHALF],
                        q_all[0][:, j, ts(i, 128)],
                        k0_sb[:, j, ts(n, DHALF)],
                        start=(j == 0),
                        stop=(j == DC - 1),
                    )
                nc.vector.tensor_copy(t[:], p[:, :DHALF])
            a_lo[0].append(tl)
            a_hi[0].append(th)

        # ---- heads 1, 2: own half of A + one background pairwise AllGather ----
        ah_dr = dram_pool.tile([2 * D, DHALF], BF16, tag="ah_dr")
        gth_dr = dram_pool.tile([4 * D, DHALF], BF16, tag="gth")
        for h in (1, 2):
            for i in range(DC):
                pool = ps_p if i % 2 == 0 else ps_r
                p = pool.tile([128, 512], F32, tag=pool.name)
                for j in range(DC):
                    nc.tensor.matmul(
                        p[:, :DHALF],
                        q_all[h][:, j, ts(i, 128)],
                        kh_sb[h - 1][:, j, :],
                        start=(j == 0),
                        stop=(j == DC - 1),
                    )
                ao = a_pool.tile([128, DHALF], BF16, tag=f"ao{h}_{i}")
                nc.vector.tensor_copy(ao[:], p[:, :DHALF])
                nc.sync.dma_start(ah_dr[ds((h - 1) * D + 128 * i, 128), :], ao[:])
        nc.gpsimd.collective_compute(
            "AllGather",
            mybir.AluOpType.bypass,
            replica_groups=PAIR_GROUPS,
            ins=[ah_dr[:].opt()],
            outs=[gth_dr[:].opt()],
        )
        # gathered rows: [rank, h-1, d] -> rank r half at rows 2*D*r + (h-1)*D
        for h in (1, 2):
            for i in range(DC):
                tl = a_pool.tile([128, DHALF], BF16, tag=f"alo{h}_{i}")
                nc.sync.dma_start(tl[:], gth_dr[ds((h - 1) * D + 128 * i, 128), :])
                a_lo[h].append(tl)
                th_ = a_pool.tile([128, DHALF], BF16, tag=f"ahi{h}_{i}")
                nc.sync.dma_start(
                    th_[:], gth_dr[ds(2 * D + (h - 1) * D + 128 * i, 128), :]
                )
                a_hi[h].append(th_)

        # out accumulators: per batch, one [128, 192] f32 tile per l-block
        out_sb = [[], []]
        for bi in range(BPC):
            for lb in range(LB):
                ot = out_pool.tile([128, HPC * DH], F32, tag=f"out{bi}_{lb}")
                out_sb[bi].append(ot)

        def a_slice(h, j, i):
            """lhsT chunk [d-chunk j, d'-slice i] of head h's A."""
            half = a_lo[h] if i < 3 else a_hi[h]
            return half[j][:, ts(i % 3, 128)]

        def emit_r(pt_sb, bi, h):
            # R^T per l-block + fused sums -> normalize (+ final out DMA)
            for lb in range(LB):
                pr = ps_r.tile([128, DHA], F32, tag=ps_r.name)
                for mj in range(LB):
                    nc.tensor.matmul(
                        pr[:],
                        pt_sb[mj][:, ts(lb, 128)],
                        vt[bi][mj][:, ds(DHA * h, DHA)],
                        start=(mj == 0),
                        stop=(mj == LB - 1),
                    )
                recip = soft_pool.tile([128, 1], F32, tag="recip")
                nc.vector.reciprocal(recip[:], pr[:, DH : DH + 1])
                nc.vector.tensor_scalar_mul(
                    out_sb[bi][lb][:, ts(h, DH)], pr[:, :DH], recip[:]
                )
                if h == HPC - 1:
                    nc.sync.dma_start(
                        out_ext[bi, ts(lb, 128), :], out_sb[bi][lb][:]
                    )

        pending_r = None
        for h in range(HPC):
            for bi in range(BPC):
                xtb = xt[bi]
                # ---- WT[d', l] = sum_d A[d,d'] xT[d,l] ----
                # The previous unit's R-block is emitted after WT's first
                # i-group so the PE fills the exp-tail latency with WT work.
                wt_sb = []
                for i in range(DC):
                    t = wt_pool.tile([128, L], BF16, tag=f"wt{i}")
                    for n in range(2):
                        p = ps_p.tile([128, 512], F32, tag=ps_p.name)
                        for j in range(DC):
                            nc.tensor.matmul(
                                p[:],
                                a_slice(h, j, i),
                                xtb[:, j, ts(n, 512)],
                                start=(j == 0),
                                stop=(j == DC - 1),
                            )
                        nc.vector.tensor_copy(t[:, ts(n, 512)], p[:])
                    wt_sb.append(t)
                    if i == 0 and pending_r is not None:
                        emit_r(*pending_r)
                        pending_r = None

                if h == 0 and bi == 1:
                    build_vt(1)  # xt[b1] has long arrived by now

                # ---- scoresT blocks + exp (pipelined over mj) ----
                def scores_t(mj):
                    p = ps_s.tile([128, L], F32, tag="ps_s")
                    for n in range(2):
                        for j in range(DC):
                            nc.tensor.matmul(
                                p[:, ts(n, 512)],
                                xtb[:, j, ts(mj, 128)],
                                wt_sb[j][:, ts(n, 512)],
                                start=(j == 0),
                                stop=(j == DC - 1),
                            )
                    return p

                pt_sb = []
                ps_prev = scores_t(0)
                for mj in range(LB):
                    ps_cur = ps_prev
                    if mj + 1 < LB:
                        ps_prev = scores_t(mj + 1)
                    pt = pt_pool.tile([128, L], BF16, tag=f"pt{mj}")
                    nc.scalar.activation(
                        pt[:], ps_cur[:], mybir.ActivationFunctionType.Exp
                    )
                    pt_sb.append(pt)
                pending_r = (pt_sb, bi, h)

        emit_r(*pending_r)

    nc.compile()
    return nc


def kernel(x, k, q, v):
    global _COMPILED
    if _COMPILED is None:
        _COMPILED = _build()

    x = np.ascontiguousarray(x, dtype=np.float32)
    k = np.ascontiguousarray(k, dtype=np.float32)
    q = np.ascontiguousarray(q, dtype=np.float32)
    v = np.ascontiguousarray(v, dtype=np.float32)

    bf = ml_dtypes.bfloat16
    xb = x.transpose(0, 2, 1).astype(bf)   # [B, D, L]
    qb = q.astype(bf)
    kb = k.astype(bf)
    vb = v.transpose(2, 0, 1).astype(bf)   # [D, H, DH]
    in_maps = []
    for c in range(N_CORES):
        bp, t = c // 4, c % 4
        hs = slice(HPC * t, HPC * (t + 1))
        h0 = HPC * t
        cols = slice(DHALF * bp, DHALF * (bp + 1))
        in_maps.append(
            {
                "xT": np.ascontiguousarray(xb[BPC * bp : BPC * (bp + 1)]),
                "q3": np.ascontiguousarray(qb[hs]),
                "k0": np.ascontiguousarray(kb[h0]),
                "kh": np.ascontiguousarray(kb[h0 + 1 : h0 + 3, :, cols]),
                "vT3": np.ascontiguousarray(vb[:, hs].reshape(D, HPC * DH)),
            }
        )

    res = run_bass_kernel_spmd(_COMPILED, in_maps, core_ids=list(range(N_CORES)))

    out = np.empty((B, L, D), np.float32)
    for c in range(N_CORES):
        bp, t = c // 4, c % 4
        for bi in range(BPC):
            out[BPC * bp + bi, :, HPC * DH * t : HPC * DH * (t + 1)] = res.results[
                c
            ]["out_r"][bi]
    return out


if __name__ == "__main__":
    rng = np.random.default_rng(0)
    x = rng.standard_normal((B, L, D)).astype(np.float32)
    k = (rng.random((H, D, D)) / D).astype(np.float32)
    q = (rng.random((H, D, D)) / D).astype(np.float32)
    v = (rng.random((H, DH, D)) / D).astype(np.float32)
    o = kernel(x=x, k=k, q=q, v=v)
    print("out", o.shape, o.dtype)



# revision 5
# speedup vs baseline: 1.2173x; 1.2173x over previous
"""Trainium2 Bass kernel for nn_Attention (B=4, L=1024, D=768, H=12, DH=64).

Reference per (batch b, head n):
    A = q_n^T k_n                [D, D]
    scores = x A x^T             [L, L]
    S = softmax(scores, -1)
    V = v_n @ x_b^T              [DH, L]
    out[b, l, n*DH+e] = sum_m S[l, m] V[e, m] / sum_m S[l, m]

Sharding: 48 (b, n) units over 8 cores; core c owns batch pair c//4 and
head triple c%4.  A is computed once per head, reused for both batches;
heads 1,2 of each triple are split column-wise across the core pair and
exchanged with a 2-rank AllGather that overlaps head-0 compute.

fp8 acceleration: A is decomposed as abar*J + Atil (abar = mean, J =
ones).  q, k are uniform-positive so A is concentrated around its mean
-- quantizing it directly to e4m3 biases the dominant rank-1 logit term
abar*S(x)S(x)^T and blows the error budget.  Instead the zero-centered
Atil runs through fp8e4 DoubleRow matmuls (2 contraction chunks per
instruction, both operands fp8) for A, WT = Atil^T x^T and the scores,
while the exact rank-1 term is injected per scores-psum with a K=1
float32r matmul (lhsT = abar*2^15*S row, rhs = S row, host-computed in
f64).  The V/R path stays bf16 (V errors land directly in the output).

Scales (e4m3 max is 240 on TRN -- values above become Inf):
    q8, k8, v-host   x512
    A psum           = A_q8 * 2^18;  at = (psum - mean)*2^2 = Atil*2^20
    WT psum          = Atil*2^20 x;  wt = psum * 2^-5 = Wtil*2^15
    scores psum      = 2^15 * (x Atil x^T + abar S S^T);  exp scale 2^-15
"""

from contextlib import ExitStack

import ml_dtypes
import numpy as np

import concourse.tile as tile
from concourse import bacc, mybir
from concourse.bass import ts, ds
from concourse.bass_utils import run_bass_kernel_spmd

# If BASS_TRACE is set in an environment that lacks antenv.axon_hooks,
# run_bass_kernel_spmd's trace path would fail on import; register a
# fallback holder so tracing degrades gracefully instead.
try:
    import antenv.axon_hooks  # noqa: F401
except Exception:  # pragma: no cover
    import sys
    import types

    import antenv

    _m = types.ModuleType("antenv.axon_hooks")
    _m._hook = None
    _m.set_axon_ntff_profile_hook = lambda h: setattr(_m, "_hook", h)
    _m.get_axon_ntff_profile_hook = lambda: _m._hook
    sys.modules["antenv.axon_hooks"] = _m
    antenv.axon_hooks = _m

B, L, D, H = 4, 1024, 768, 12
DH = D // H          # 64
HPC = 3              # heads per core
BPC = 2              # batches per core
N_CORES = 8
DC = D // 128        # 6 chunks of the contraction/feature dim
JP = DC // 2         # 3 DoubleRow chunk-pairs
LB = L // 128        # 8 l-blocks / m-blocks
DHA = DH + 1         # 65: head slice width in vt_aug (ones column at 64)
DHALF = D // 2       # 384: A-half width for the pairwise exchange
F32 = mybir.dt.float32
F32R = mybir.dt.float32r
BF16 = mybir.dt.bfloat16
F8 = mybir.dt.float8e4
DR = mybir.MatmulPerfMode.DoubleRow
PAIR_GROUPS = [[0, 4], [1, 5], [2, 6], [3, 7]]

SQK = 512.0          # host scale on q, k
A_SC = 4.0           # at = (A_psum - mean) * 2^2  -> Atil * 2^20
W_SC = 2.0**-5       # wt = WT_psum * 2^-5         -> Wtil * 2^15
S_SC = 2.0**15       # scores psum scale; exp uses 1/S_SC
N_WARMUP = 28        # dummy matmuls to lift the HAM clock gate early

_COMPILED = None


def _build():
    nc = bacc.Bacc(
        "TRN2",
        target_bir_lowering=False,
        debug=False,
        enable_asserts=False,
        num_devices=N_CORES,
    )
    xT8_ext = nc.dram_tensor("xT8", [BPC, D, L], F8, kind="ExternalInput").ap()
    xTb_ext = nc.dram_tensor("xTb", [BPC, D, L], BF16, kind="ExternalInput").ap()
    q3_ext = nc.dram_tensor("q3", [HPC, D, D], F8, kind="ExternalInput").ap()
    k0_ext = nc.dram_tensor("k0", [D, D], F8, kind="ExternalInput").ap()
    kh_ext = nc.dram_tensor("kh", [2, D, DHALF], F8, kind="ExternalInput").ap()
    vT3_ext = nc.dram_tensor("vT3", [D, HPC * DH], BF16, kind="ExternalInput").ap()
    smh_ext = nc.dram_tensor("smh", [HPC, BPC, L], F32R, kind="ExternalInput").ap()
    smp_ext = nc.dram_tensor("smp", [BPC, L], F32R, kind="ExternalInput").ap()
    csub_ext = nc.dram_tensor("csub", [128, HPC], F32, kind="ExternalInput").ap()
    out_ext = nc.dram_tensor(
        "out_r", [BPC, L, HPC * DH], F32, kind="ExternalOutput"
    ).ap()

    with tile.TileContext(nc) as tc, ExitStack() as ctx:
        xt_pool = ctx.enter_context(tc.tile_pool(name="xt", bufs=1))
        vt3_pool = ctx.enter_context(tc.tile_pool(name="vt3", bufs=1))
        vt_pool = ctx.enter_context(tc.tile_pool(name="vt", bufs=1))
        qk_pool = ctx.enter_context(tc.tile_pool(name="qk", bufs=1))
        a_pool = ctx.enter_context(tc.tile_pool(name="a", bufs=1))
        wt_pool = ctx.enter_context(tc.tile_pool(name="wt", bufs=2))
        pt_pool = ctx.enter_context(tc.tile_pool(name="pt", bufs=2))
        soft_pool = ctx.enter_context(tc.tile_pool(name="soft", bufs=2))
        out_pool = ctx.enter_context(tc.tile_pool(name="outp", bufs=1))
        sm_pool = ctx.enter_context(tc.tile_pool(name="sm", bufs=1))
        dram_pool = ctx.enter_context(tc.tile_pool(name="dram", bufs=1, space="DRAM"))
        ps_p = ctx.enter_context(tc.tile_pool(name="ps_p", bufs=2, space="PSUM"))
        ps_s = ctx.enter_context(tc.tile_pool(name="ps_s", bufs=2, space="PSUM"))
        ps_r = ctx.enter_context(tc.tile_pool(name="ps_r", bufs=2, space="PSUM"))

        # ---------- HAM warmup: keep the PE busy from t=0 so the clock
        # gate lifts (~3.4us) before the real matmul stream begins.
        dummy = sm_pool.tile([128, 128], F8, tag="dummy")
        nc.gpsimd.memset(dummy[:], 0.0)
        for w in range(N_WARMUP):
            pw = ps_p.tile([128, 512], F32, tag=ps_p.name)
            nc.tensor.matmul(pw[:, :128], dummy[:], dummy[:], start=True, stop=True)

        # ---------- loads ----------
        def load3d(pool, tag, dram2d, width, dtype):
            t = pool.tile([128, DC, width], dtype, tag=tag)
            nc.sync.dma_start(t[:], dram2d.rearrange("(c p) w -> p c w", p=128))
            return t

        q_all = [None, None, None]
        q_all[0] = load3d(qk_pool, "q0", q3_ext[0], D, F8)
        k0_sb = load3d(qk_pool, "k0", k0_ext[:], D, F8)
        csub_sb = sm_pool.tile([128, HPC], F32, tag="csub")
        nc.sync.dma_start(csub_sb[:], csub_ext[:])
        smh_t = [[None] * BPC for _ in range(HPC)]
        smp_t = [None, None]
        for bi in range(BPC):
            t_smp = sm_pool.tile([1, L], F32R, tag=f"smp{bi}")
            nc.sync.dma_start(t_smp[:], smp_ext[ds(bi, 1), :])
            smp_t[bi] = t_smp
            for h in range(HPC):
                t_smh = sm_pool.tile([1, L], F32R, tag=f"smh{h}_{bi}")
                nc.sync.dma_start(t_smh[:], smh_ext[h, ds(bi, 1), :])
                smh_t[h][bi] = t_smh
        xt8 = [None, None]
        xtb = [None, None]
        xt8[0] = load3d(xt_pool, "x8_0", xT8_ext[0], L, F8)
        xtb[0] = load3d(xt_pool, "xb_0", xTb_ext[0], L, BF16)
        vt3 = load3d(vt3_pool, "vt3", vT3_ext[:], HPC * DH, BF16)
        xt8[1] = load3d(xt_pool, "x8_1", xT8_ext[1], L, F8)
        kh_sb = [None, None]
        for h in (1, 2):
            q_all[h] = load3d(qk_pool, f"q{h}", q3_ext[h], D, F8)
            kh_sb[h - 1] = load3d(qk_pool, f"kh{h}", kh_ext[h - 1], DHALF, F8)
        xtb[1] = load3d(xt_pool, "xb_1", xTb_ext[1], L, BF16)

        # at[h][p, j, d'] = Atil[128j+p, d'] * 2^20 in fp8
        at = [
            a_pool.tile([128, DC, D], F8, tag=f"at{h}", name=f"at{h}")
            for h in range(HPC)
        ]

        # ---- head 0: full Atil locally ----
        for i in range(DC):
            for n in range(2):
                pool = ps_p if n == 0 else ps_r
                p = pool.tile([128, 512], F32, tag=pool.name)
                for j in range(JP):
                    nc.tensor.matmul(
                        p[:, :DHALF],
                        q_all[0][:, 2 * j : 2 * j + 2, ts(i, 128)],
                        k0_sb[:, 2 * j : 2 * j + 2, ds(n * DHALF, DHALF)],
                        start=(j == 0),
                        stop=(j == JP - 1),
                        perf_mode=DR,
                    )
                nc.vector.tensor_scalar(
                    at[0][:, i, ds(n * DHALF, DHALF)],
                    p[:, :DHALF],
                    csub_sb[:, 0:1],
                    A_SC,
                    op0=mybir.AluOpType.subtract,
                    op1=mybir.AluOpType.mult,
                )

        # ---- heads 1, 2: own half of Atil + background pairwise AllGather ----
        ah_dr = dram_pool.tile([2 * D, DHALF], F8, tag="ah_dr")
        gth_dr = dram_pool.tile([4 * D, DHALF], F8, tag="gth")
        for h in (1, 2):
            for i in range(DC):
                pool = ps_p if i % 2 == 0 else ps_r
                p = pool.tile([128, 512], F32, tag=pool.name)
                for j in range(JP):
                    nc.tensor.matmul(
                        p[:, :DHALF],
                        q_all[h][:, 2 * j : 2 * j + 2, ts(i, 128)],
                        kh_sb[h - 1][:, 2 * j : 2 * j + 2, :],
                        start=(j == 0),
                        stop=(j == JP - 1),
                        perf_mode=DR,
                    )
                ao = a_pool.tile([128, DHALF], F8, tag=f"ao{h}_{i}")
                nc.vector.tensor_scalar(
                    ao[:],
                    p[:, :DHALF],
                    csub_sb[:, h : h + 1],
                    A_SC,
                    op0=mybir.AluOpType.subtract,
                    op1=mybir.AluOpType.mult,
                )
                nc.sync.dma_start(ah_dr[ds((h - 1) * D + 128 * i, 128), :], ao[:])
        nc.gpsimd.collective_compute(
            "AllGather",
            mybir.AluOpType.bypass,
            replica_groups=PAIR_GROUPS,
            ins=[ah_dr[:].opt()],
            outs=[gth_dr[:].opt()],
        )
        # gathered rows: [rank, h-1, d]; rank 0 owns cols 0:384, rank 1 the rest
        for h in (1, 2):
            for i in range(DC):
                nc.sync.dma_start(
                    at[h][:, i, ds(0, DHALF)],
                    gth_dr[ds((h - 1) * D + 128 * i, 128), :],
                )
                nc.sync.dma_start(
                    at[h][:, i, ds(DHALF, DHALF)],
                    gth_dr[ds(2 * D + (h - 1) * D + 128 * i, 128), :],
                )

        # ---------- VT_aug projection per batch (bf16, as V errors are
        # first-order in the output) ----------
        vt = [None, None]

        def build_vt(bi):
            tiles = []
            for j in range(LB):
                pool = ps_p if j % 2 == 0 else ps_r
                p = pool.tile([128, 512], F32, tag=pool.name)
                for i in range(DC):
                    nc.tensor.matmul(
                        p[:, : HPC * DH],
                        xtb[bi][:, i, ts(j, 128)],
                        vt3[:, i, :],
                        start=(i == 0),
                        stop=(i == DC - 1),
                    )
                t = vt_pool.tile([128, HPC * DHA], BF16, tag=f"vt{bi}_{j}")
                nc.gpsimd.memset(t[:], 1.0)
                t3 = t[:].rearrange("p (h c) -> p h c", h=HPC)
                p3 = p[:, : HPC * DH].rearrange("p (h c) -> p h c", h=HPC)
                nc.vector.tensor_copy(t3[:, :, :DH], p3[:])
                tiles.append(t)
            vt[bi] = tiles

        build_vt(0)

        # out accumulators: per batch, one [128, 192] f32 tile per l-block
        out_sb = [[], []]
        for bi in range(BPC):
            for lb in range(LB):
                ot = out_pool.tile([128, HPC * DH], F32, tag=f"out{bi}_{lb}")
                out_sb[bi].append(ot)

        def emit_r(pt_sb, bi, h):
            # R per l-block + fused sums -> normalize (+ final out DMA)
            for lb in range(LB):
                pr = ps_r.tile([128, DHA], F32, tag=ps_r.name)
                for mj in range(LB):
                    nc.tensor.matmul(
                        pr[:],
                        pt_sb[mj][:, ts(lb, 128)],
                        vt[bi][mj][:, ds(DHA * h, DHA)],
                        start=(mj == 0),
                        stop=(mj == LB - 1),
                    )
                recip = soft_pool.tile([128, 1], F32, tag="recip")
                nc.vector.reciprocal(recip[:], pr[:, DH : DH + 1])
                nc.vector.tensor_scalar_mul(
                    out_sb[bi][lb][:, ts(h, DH)], pr[:, :DH], recip[:]
                )
                if h == HPC - 1:
                    nc.sync.dma_start(
                        out_ext[bi, ts(lb, 128), :], out_sb[bi][lb][:]
                    )

        pending_r = None
        for h in range(HPC):
            for bi in range(BPC):
                # ---- WT[d', l] = sum_d Atil[d, d'] xT[d, l] * 2^20 ----
                wt = wt_pool.tile([128, DC, L], F8, tag="wt")
                for i in range(DC):
                    for n in range(2):
                        p = ps_p.tile([128, 512], F32, tag=ps_p.name)
                        for j in range(JP):
                            nc.tensor.matmul(
                                p[:],
                                at[h][:, 2 * j : 2 * j + 2, ts(i, 128)],
                                xt8[bi][:, 2 * j : 2 * j + 2, ts(n, 512)],
                                start=(j == 0),
                                stop=(j == JP - 1),
                                perf_mode=DR,
                            )
                        nc.vector.tensor_scalar_mul(
                            wt[:, i, ts(n, 512)], p[:], W_SC
                        )
                    if i == 0 and pending_r is not None:
                        emit_r(*pending_r)
                        pending_r = None

                if h == 0 and bi == 1:
                    build_vt(1)

                # ---- scoresT blocks: rank-1 preload + fp8 DR accumulate ----
                def scores_t(mj):
                    p = ps_s.tile([128, L], F32, tag="ps_s")
                    for n in range(2):
                        nc.tensor.matmul(
                            p[:, ts(n, 512)],
                            smh_t[h][bi][:, ts(mj, 128)],
                            smp_t[bi][:, ts(n, 512)],
                            start=True,
                            stop=False,
                            skip_group_check=True,
                        )
                        for j in range(JP):
                            nc.tensor.matmul(
                                p[:, ts(n, 512)],
                                xt8[bi][:, 2 * j : 2 * j + 2, ts(mj, 128)],
                                wt[:, 2 * j : 2 * j + 2, ts(n, 512)],
                                start=False,
                                stop=(j == JP - 1),
                                perf_mode=DR,
                                skip_group_check=True,
                            )
                    return p

                pt_sb = []
                ps_prev = scores_t(0)
                for mj in range(LB):
                    ps_cur = ps_prev
                    if mj + 1 < LB:
                        ps_prev = scores_t(mj + 1)
                    pt = pt_pool.tile([128, L], BF16, tag=f"pt{mj}")
                    nc.scalar.activation(
                        pt[:],
                        ps_cur[:],
                        mybir.ActivationFunctionType.Exp,
                        scale=1.0 / S_SC,
                    )
                    pt_sb.append(pt)
                pending_r = (pt_sb, bi, h)

        emit_r(*pending_r)

    nc.compile()
    return nc


def kernel(x, k, q, v):
    global _COMPILED
    if _COMPILED is None:
        _COMPILED = _build()

    x = np.ascontiguousarray(x, dtype=np.float32)
    k = np.ascontiguousarray(k, dtype=np.float32)
    q = np.ascontiguousarray(q, dtype=np.float32)
    v = np.ascontiguousarray(v, dtype=np.float32)

    bf = ml_dtypes.bfloat16
    f8 = ml_dtypes.float8_e4m3
    xT = x.transpose(0, 2, 1)              # [B, D, L]
    xT8 = xT.astype(f8)
    xTb = xT.astype(bf)
    q8 = (q * SQK).astype(f8)
    k8 = (k * SQK).astype(f8)
    vb = v.transpose(2, 0, 1).astype(bf)   # [D, H, DH]

    # exact rank-1 pieces (f64): abar = mean(q^T k), S = row sums of x
    q64 = q.astype(np.float64)
    k64 = k.astype(np.float64)
    abar = (q64.sum(axis=2) * k64.sum(axis=2)).sum(axis=1) / (D * D)  # [H]
    S = x.astype(np.float64).sum(axis=2)                              # [B, L]
    # mean of the *quantized* A (what the device must subtract)
    q8f = q8.astype(np.float64)
    k8f = k8.astype(np.float64)
    c_ps = (q8f.sum(axis=2) * k8f.sum(axis=2)).sum(axis=1) / (D * D)  # [H]
    smh = (abar[:, None, None] * S_SC * S[None]).astype(np.float32)   # [H, B, L]
    smp = S.astype(np.float32)                                        # [B, L]

    in_maps = []
    for c in range(N_CORES):
        bp, t = c // 4, c % 4
        hs = slice(HPC * t, HPC * (t + 1))
        h0 = HPC * t
        bsl = slice(BPC * bp, BPC * (bp + 1))
        cols = slice(DHALF * bp, DHALF * (bp + 1))
        in_maps.append(
            {
                "xT8": np.ascontiguousarray(xT8[bsl]),
                "xTb": np.ascontiguousarray(xTb[bsl]),
                "q3": np.ascontiguousarray(q8[hs]),
                "k0": np.ascontiguousarray(k8[h0]),
                "kh": np.ascontiguousarray(k8[h0 + 1 : h0 + 3, :, cols]),
                "vT3": np.ascontiguousarray(vb[:, hs].reshape(D, HPC * DH)),
                "smh": np.ascontiguousarray(smh[hs, bsl]),
                "smp": np.ascontiguousarray(smp[bsl]),
                "csub": np.ascontiguousarray(
                    np.broadcast_to(
                        c_ps[hs].astype(np.float32)[None, :], (128, HPC)
                    )
                ),
            }
        )

    res = run_bass_kernel_spmd(_COMPILED, in_maps, core_ids=list(range(N_CORES)))

    out = np.empty((B, L, D), np.float32)
    for c in range(N_CORES):
        bp, t = c // 4, c % 4
        for bi in range(BPC):
            out[BPC * bp + bi, :, HPC * DH * t : HPC * DH * (t + 1)] = res.results[
                c
            ]["out_r"][bi]
    return out


if __name__ == "__main__":
    rng = np.random.default_rng(0)
    x = rng.standard_normal((B, L, D)).astype(np.float32)
    k = (rng.random((H, D, D)) / D).astype(np.float32)
    q = (rng.random((H, D, D)) / D).astype(np.float32)
    v = (rng.random((H, DH, D)) / D).astype(np.float32)
    o = kernel(x=x, k=k, q=q, v=v)
    print("out", o.shape, o.dtype)


# revision 6
# speedup vs baseline: 1.5168x; 1.2460x over previous
"""Trainium2 Bass kernel for nn_Attention (B=4, L=1024, D=768, H=12, DH=64).

Reference per (batch b, head n):
    A = q_n^T k_n                [D, D]
    scores = x A x^T             [L, L]
    S = softmax(scores, -1)
    V = v_n @ x_b^T              [DH, L]
    out[b, l, n*DH+e] = sum_m S[l, m] V[e, m] / sum_m S[l, m]

Sharding: 48 (b, n) units over 8 cores; core c owns batch pair c//4 and
head triple c%4.  A is computed once per head, reused for both batches;
heads 1,2 of each triple are split column-wise across the core pair and
exchanged with a 2-rank AllGather that overlaps head-0 compute.

fp8 acceleration: A is decomposed as abar*J + Atil (abar = mean, J =
ones).  q, k are uniform-positive so A is concentrated around its mean
-- quantizing it directly to e4m3 biases the dominant rank-1 logit term
abar*S(x)S(x)^T and blows the error budget.  Instead the zero-centered
Atil runs through fp8e4 DoubleRow matmuls (2 contraction chunks per
instruction, both operands fp8) for A, WT = Atil^T x^T and the scores,
while the exact rank-1 term is injected per scores-psum with a K=1
float32r matmul (lhsT = abar*2^15*S row, rhs = S row, host-computed in
f64).  The V/R path stays bf16 (V errors land directly in the output).

Scales (e4m3 max is 240 on TRN -- values above become Inf):
    q8, k8, v-host   x512
    A psum           = A_q8 * 2^18;  at = (psum - mean)*2^2 = Atil*2^20
    WT psum          = Atil*2^20 x;  wt = psum * 2^-5 = Wtil*2^15
    scores psum      = 2^15 * (x Atil x^T + abar S S^T);  exp scale 2^-15
"""

from contextlib import ExitStack

import ml_dtypes
import numpy as np

import concourse.tile as tile
from concourse import bacc, mybir
from concourse.bass import ts, ds
from concourse.bass_utils import run_bass_kernel_spmd

# If BASS_TRACE is set in an environment that lacks antenv.axon_hooks,
# run_bass_kernel_spmd's trace path would fail on import; register a
# fallback holder so tracing degrades gracefully instead.
try:
    import antenv.axon_hooks  # noqa: F401
except Exception:  # pragma: no cover
    import sys
    import types

    import antenv

    _m = types.ModuleType("antenv.axon_hooks")
    _m._hook = None
    _m.set_axon_ntff_profile_hook = lambda h: setattr(_m, "_hook", h)
    _m.get_axon_ntff_profile_hook = lambda: _m._hook
    sys.modules["antenv.axon_hooks"] = _m
    antenv.axon_hooks = _m

B, L, D, H = 4, 1024, 768, 12
DH = D // H          # 64
HPC = 3              # heads per core
BPC = 2              # batches per core
N_CORES = 8
DC = D // 128        # 6 chunks of the contraction/feature dim
JP = DC // 2         # 3 DoubleRow chunk-pairs
LB = L // 128        # 8 l-blocks / m-blocks
DHA = DH + 1         # 65: head slice width in vt_aug (ones column at 64)
DHALF = D // 2       # 384: A-half width for the pairwise exchange
F32 = mybir.dt.float32
F32R = mybir.dt.float32r
BF16 = mybir.dt.bfloat16
F8 = mybir.dt.float8e4
DR = mybir.MatmulPerfMode.DoubleRow
PAIR_GROUPS = [[0, 4], [1, 5], [2, 6], [3, 7]]

SQK = 512.0          # host scale on q, k
A_SC = 4.0           # at = (A_psum - mean) * 2^2  -> Atil * 2^20
W_SC = 2.0**-5       # wt = WT_psum * 2^-5         -> Wtil * 2^15
S_SC = 2.0**15       # scores psum scale; exp uses 1/S_SC
N_WARMUP = 28        # dummy matmuls to lift the HAM clock gate early

_COMPILED = None


def _build():
    nc = bacc.Bacc(
        "TRN2",
        target_bir_lowering=False,
        debug=False,
        enable_asserts=False,
        num_devices=N_CORES,
    )
    xT8_ext = nc.dram_tensor("xT8", [BPC, D, L], F8, kind="ExternalInput").ap()
    xTb_ext = nc.dram_tensor("xTb", [BPC, D, L], BF16, kind="ExternalInput").ap()
    q3_ext = nc.dram_tensor("q3", [HPC, D, D], F8, kind="ExternalInput").ap()
    k0_ext = nc.dram_tensor("k0", [D, D], F8, kind="ExternalInput").ap()
    kh_ext = nc.dram_tensor("kh", [2, D, DHALF], F8, kind="ExternalInput").ap()
    vT3_ext = nc.dram_tensor("vT3", [D, HPC * DH], BF16, kind="ExternalInput").ap()
    smh_ext = nc.dram_tensor("smh", [HPC, BPC, L], BF16, kind="ExternalInput").ap()
    smp_ext = nc.dram_tensor("smp", [BPC, L], BF16, kind="ExternalInput").ap()
    cneg_ext = nc.dram_tensor("cneg", [128, HPC], F32, kind="ExternalInput").ap()
    out_ext = nc.dram_tensor(
        "out_r", [BPC, L, HPC * DH], F32, kind="ExternalOutput"
    ).ap()

    with tile.TileContext(nc) as tc, ExitStack() as ctx:
        xt_pool = ctx.enter_context(tc.tile_pool(name="xt", bufs=1))
        vt3_pool = ctx.enter_context(tc.tile_pool(name="vt3", bufs=1))
        vt_pool = ctx.enter_context(tc.tile_pool(name="vt", bufs=1))
        qk_pool = ctx.enter_context(tc.tile_pool(name="qk", bufs=1))
        a_pool = ctx.enter_context(tc.tile_pool(name="a", bufs=1))
        wt_pool = ctx.enter_context(tc.tile_pool(name="wt", bufs=2))
        pt_pool = ctx.enter_context(tc.tile_pool(name="pt", bufs=2))
        soft_pool = ctx.enter_context(tc.tile_pool(name="soft", bufs=2))
        out_pool = ctx.enter_context(tc.tile_pool(name="outp", bufs=1))
        sm_pool = ctx.enter_context(tc.tile_pool(name="sm", bufs=1))
        dram_pool = ctx.enter_context(tc.tile_pool(name="dram", bufs=1, space="DRAM"))
        ps_p = ctx.enter_context(tc.tile_pool(name="ps_p", bufs=2, space="PSUM"))
        ps_s = ctx.enter_context(tc.tile_pool(name="ps_s", bufs=2, space="PSUM"))
        ps_r = ctx.enter_context(tc.tile_pool(name="ps_r", bufs=2, space="PSUM"))

        # ---------- HAM warmup: keep the PE busy from t=0 so the clock
        # gate lifts (~3.4us) before the real matmul stream begins.
        dummy = sm_pool.tile([128, 128], F8, tag="dummy")
        nc.gpsimd.memset(dummy[:], 0.0)
        for w in range(N_WARMUP):
            pw = ps_p.tile([128, 512], F32, tag=ps_p.name)
            nc.tensor.matmul(pw[:, :128], dummy[:], dummy[:], start=True, stop=True)

        # ---------- loads ----------
        def load3d(pool, tag, dram2d, width, dtype):
            t = pool.tile([128, DC, width], dtype, tag=tag)
            nc.sync.dma_start(t[:], dram2d.rearrange("(c p) w -> p c w", p=128))
            return t

        q_all = [None, None, None]
        q_all[0] = load3d(qk_pool, "q0", q3_ext[0], D, F8)
        k0_sb = load3d(qk_pool, "k0", k0_ext[:], D, F8)
        cneg_sb = sm_pool.tile([128, HPC], F32, tag="cneg")
        nc.sync.dma_start(cneg_sb[:], cneg_ext[:])
        smh_t = [[None] * BPC for _ in range(HPC)]
        smp_t = [None, None]
        for bi in range(BPC):
            t_smp = sm_pool.tile([1, L], BF16, tag=f"smp{bi}")
            nc.sync.dma_start(t_smp[:], smp_ext[ds(bi, 1), :])
            smp_t[bi] = t_smp
            for h in range(HPC):
                t_smh = sm_pool.tile([1, L], BF16, tag=f"smh{h}_{bi}")
                nc.sync.dma_start(t_smh[:], smh_ext[h, ds(bi, 1), :])
                smh_t[h][bi] = t_smh
        xt8 = [None, None]
        xtb = [None, None]
        xt8[0] = load3d(xt_pool, "x8_0", xT8_ext[0], L, F8)
        xtb[0] = load3d(xt_pool, "xb_0", xTb_ext[0], L, BF16)
        vt3 = load3d(vt3_pool, "vt3", vT3_ext[:], HPC * DH, BF16)
        xt8[1] = load3d(xt_pool, "x8_1", xT8_ext[1], L, F8)
        kh_sb = [None, None]
        for h in (1, 2):
            q_all[h] = load3d(qk_pool, f"q{h}", q3_ext[h], D, F8)
            kh_sb[h - 1] = load3d(qk_pool, f"kh{h}", kh_ext[h - 1], DHALF, F8)
        xtb[1] = load3d(xt_pool, "xb_1", xTb_ext[1], L, BF16)

        # at[h][p, j, d'] = Atil[128j+p, d'] * 2^20 in fp8
        at = [
            a_pool.tile([128, DC, D], F8, tag=f"at{h}", name=f"at{h}")
            for h in range(HPC)
        ]

        # ---- head 0: full Atil locally ----
        for i in range(DC):
            for n in range(2):
                pool = ps_p if n == 0 else ps_r
                p = pool.tile([128, 512], F32, tag=pool.name)
                for j in range(JP):
                    nc.tensor.matmul(
                        p[:, :DHALF],
                        q_all[0][:, 2 * j : 2 * j + 2, ts(i, 128)],
                        k0_sb[:, 2 * j : 2 * j + 2, ds(n * DHALF, DHALF)],
                        start=(j == 0),
                        stop=(j == JP - 1),
                        perf_mode=DR,
                    )
                nc.scalar.activation(
                    at[0][:, i, ds(n * DHALF, DHALF)],
                    p[:, :DHALF],
                    mybir.ActivationFunctionType.Identity,
                    bias=cneg_sb[:, 0:1],
                    scale=A_SC,
                )

        # ---- heads 1, 2: own half of Atil + background pairwise AllGather ----
        ah_dr = dram_pool.tile([2 * D, DHALF], F8, tag="ah_dr")
        gth_dr = dram_pool.tile([4 * D, DHALF], F8, tag="gth")
        for h in (1, 2):
            for i in range(DC):
                pool = ps_p if i % 2 == 0 else ps_r
                p = pool.tile([128, 512], F32, tag=pool.name)
                for j in range(JP):
                    nc.tensor.matmul(
                        p[:, :DHALF],
                        q_all[h][:, 2 * j : 2 * j + 2, ts(i, 128)],
                        kh_sb[h - 1][:, 2 * j : 2 * j + 2, :],
                        start=(j == 0),
                        stop=(j == JP - 1),
                        perf_mode=DR,
                    )
                ao = a_pool.tile([128, DHALF], F8, tag=f"ao{h}_{i}")
                nc.scalar.activation(
                    ao[:],
                    p[:, :DHALF],
                    mybir.ActivationFunctionType.Identity,
                    bias=cneg_sb[:, h : h + 1],
                    scale=A_SC,
                )
                nc.sync.dma_start(ah_dr[ds((h - 1) * D + 128 * i, 128), :], ao[:])
        nc.gpsimd.collective_compute(
            "AllGather",
            mybir.AluOpType.bypass,
            replica_groups=PAIR_GROUPS,
            ins=[ah_dr[:].opt()],
            outs=[gth_dr[:].opt()],
        )
        # gathered rows: [rank, h-1, d]; rank 0 owns cols 0:384, rank 1 the rest
        for h in (1, 2):
            for i in range(DC):
                nc.sync.dma_start(
                    at[h][:, i, ds(0, DHALF)],
                    gth_dr[ds((h - 1) * D + 128 * i, 128), :],
                )
                nc.sync.dma_start(
                    at[h][:, i, ds(DHALF, DHALF)],
                    gth_dr[ds(2 * D + (h - 1) * D + 128 * i, 128), :],
                )

        # ---------- VT_aug projection per batch (bf16, as V errors are
        # first-order in the output) ----------
        vt = [None, None]

        def build_vt(bi):
            tiles = []
            for j in range(LB):
                pool = ps_p if j % 2 == 0 else ps_r
                p = pool.tile([128, 512], F32, tag=pool.name)
                for i in range(DC):
                    nc.tensor.matmul(
                        p[:, : HPC * DH],
                        xtb[bi][:, i, ts(j, 128)],
                        vt3[:, i, :],
                        start=(i == 0),
                        stop=(i == DC - 1),
                    )
                t = vt_pool.tile([128, HPC * DHA], BF16, tag=f"vt{bi}_{j}")
                nc.gpsimd.memset(t[:], 1.0)
                t3 = t[:].rearrange("p (h c) -> p h c", h=HPC)
                p3 = p[:, : HPC * DH].rearrange("p (h c) -> p h c", h=HPC)
                nc.vector.tensor_copy(t3[:, :, :DH], p3[:])
                tiles.append(t)
            vt[bi] = tiles

        build_vt(0)

        # out accumulators: per batch, one [128, 192] f32 tile per l-block
        out_sb = [[], []]
        for bi in range(BPC):
            for lb in range(LB):
                ot = out_pool.tile([128, HPC * DH], F32, tag=f"out{bi}_{lb}")
                out_sb[bi].append(ot)

        def emit_r(pt_sb, bi, h):
            # R per l-block + fused sums -> normalize (+ final out DMA)
            for lb in range(LB):
                pr = ps_r.tile([128, DHA], F32, tag=ps_r.name)
                for mj in range(LB):
                    nc.tensor.matmul(
                        pr[:],
                        pt_sb[mj][:, ts(lb, 128)],
                        vt[bi][mj][:, ds(DHA * h, DHA)],
                        start=(mj == 0),
                        stop=(mj == LB - 1),
                    )
                recip = soft_pool.tile([128, 1], F32, tag="recip")
                nc.vector.reciprocal(recip[:], pr[:, DH : DH + 1])
                nc.vector.tensor_scalar_mul(
                    out_sb[bi][lb][:, ts(h, DH)], pr[:, :DH], recip[:]
                )
                if h == HPC - 1:
                    nc.sync.dma_start(
                        out_ext[bi, ts(lb, 128), :], out_sb[bi][lb][:]
                    )

        pending_r = None
        for h in range(HPC):
            for bi in range(BPC):
                # ---- WT[d', l] = sum_d Atil[d, d'] xT[d, l] * 2^20 ----
                wt = wt_pool.tile([128, DC, L], F8, tag="wt")
                for i in range(DC):
                    for n in range(2):
                        p = ps_p.tile([128, 512], F32, tag=ps_p.name)
                        for j in range(JP):
                            nc.tensor.matmul(
                                p[:],
                                at[h][:, 2 * j : 2 * j + 2, ts(i, 128)],
                                xt8[bi][:, 2 * j : 2 * j + 2, ts(n, 512)],
                                start=(j == 0),
                                stop=(j == JP - 1),
                                perf_mode=DR,
                            )
                        nc.vector.tensor_scalar_mul(
                            wt[:, i, ts(n, 512)], p[:], W_SC
                        )
                    if i == 0 and pending_r is not None:
                        emit_r(*pending_r)
                        pending_r = None

                if h == 0 and bi == 1:
                    build_vt(1)

                # ---- scoresT blocks: rank-1 preload + fp8 DR accumulate ----
                def scores_t(mj):
                    p = ps_s.tile([128, L], F32, tag="ps_s")
                    for n in range(2):
                        nc.tensor.matmul(
                            p[:, ts(n, 512)],
                            smh_t[h][bi][:, ts(mj, 128)],
                            smp_t[bi][:, ts(n, 512)],
                            start=True,
                            stop=False,
                            skip_group_check=True,
                        )
                        for j in range(JP):
                            nc.tensor.matmul(
                                p[:, ts(n, 512)],
                                xt8[bi][:, 2 * j : 2 * j + 2, ts(mj, 128)],
                                wt[:, 2 * j : 2 * j + 2, ts(n, 512)],
                                start=False,
                                stop=(j == JP - 1),
                                perf_mode=DR,
                                skip_group_check=True,
                            )
                    return p

                pt_sb = []
                ps_prev = scores_t(0)
                for mj in range(LB):
                    ps_cur = ps_prev
                    if mj + 1 < LB:
                        ps_prev = scores_t(mj + 1)
                    pt = pt_pool.tile([128, L], BF16, tag=f"pt{mj}")
                    nc.scalar.activation(
                        pt[:],
                        ps_cur[:],
                        mybir.ActivationFunctionType.Exp,
                        scale=1.0 / S_SC,
                    )
                    pt_sb.append(pt)
                pending_r = (pt_sb, bi, h)

        emit_r(*pending_r)

    nc.compile()
    return nc


def kernel(x, k, q, v):
    global _COMPILED
    if _COMPILED is None:
        _COMPILED = _build()

    x = np.ascontiguousarray(x, dtype=np.float32)
    k = np.ascontiguousarray(k, dtype=np.float32)
    q = np.ascontiguousarray(q, dtype=np.float32)
    v = np.ascontiguousarray(v, dtype=np.float32)

    bf = ml_dtypes.bfloat16
    f8 = ml_dtypes.float8_e4m3
    xT = x.transpose(0, 2, 1)              # [B, D, L]
    xT8 = xT.astype(f8)
    xTb = xT.astype(bf)
    q8 = (q * SQK).astype(f8)
    k8 = (k * SQK).astype(f8)
    vb = v.transpose(2, 0, 1).astype(bf)   # [D, H, DH]

    # exact rank-1 pieces (f64): abar = mean(q^T k), S = row sums of x
    q64 = q.astype(np.float64)
    k64 = k.astype(np.float64)
    abar = (q64.sum(axis=2) * k64.sum(axis=2)).sum(axis=1) / (D * D)  # [H]
    S = x.astype(np.float64).sum(axis=2)                              # [B, L]
    # mean of the *quantized* A (what the device must subtract)
    q8f = q8.astype(np.float64)
    k8f = k8.astype(np.float64)
    c_ps = (q8f.sum(axis=2) * k8f.sum(axis=2)).sum(axis=1) / (D * D)  # [H]
    smh = (abar[:, None, None] * S_SC * S[None]).astype(bf)           # [H, B, L]
    smp = S.astype(bf)                                                # [B, L]

    in_maps = []
    for c in range(N_CORES):
        bp, t = c // 4, c % 4
        hs = slice(HPC * t, HPC * (t + 1))
        h0 = HPC * t
        bsl = slice(BPC * bp, BPC * (bp + 1))
        cols = slice(DHALF * bp, DHALF * (bp + 1))
        in_maps.append(
            {
                "xT8": np.ascontiguousarray(xT8[bsl]),
                "xTb": np.ascontiguousarray(xTb[bsl]),
                "q3": np.ascontiguousarray(q8[hs]),
                "k0": np.ascontiguousarray(k8[h0]),
                "kh": np.ascontiguousarray(k8[h0 + 1 : h0 + 3, :, cols]),
                "vT3": np.ascontiguousarray(vb[:, hs].reshape(D, HPC * DH)),
                "smh": np.ascontiguousarray(smh[hs, bsl]),
                "smp": np.ascontiguousarray(smp[bsl]),
                "cneg": np.ascontiguousarray(
                    np.broadcast_to(
                        (-A_SC * c_ps[hs]).astype(np.float32)[None, :], (128, HPC)
                    )
                ),
            }
        )

    res = run_bass_kernel_spmd(_COMPILED, in_maps, core_ids=list(range(N_CORES)))

    out = np.empty((B, L, D), np.float32)
    for c in range(N_CORES):
        bp, t = c // 4, c % 4
        for bi in range(BPC):
            out[BPC * bp + bi, :, HPC * DH * t : HPC * DH * (t + 1)] = res.results[
                c
            ]["out_r"][bi]
    return out


if __name__ == "__main__":
    rng = np.random.default_rng(0)
    x = rng.standard_normal((B, L, D)).astype(np.float32)
    k = (rng.random((H, D, D)) / D).astype(np.float32)
    q = (rng.random((H, D, D)) / D).astype(np.float32)
    v = (rng.random((H, DH, D)) / D).astype(np.float32)
    o = kernel(x=x, k=k, q=q, v=v)
    print("out", o.shape, o.dtype)


# revision 7
# speedup vs baseline: 1.5200x; 1.0021x over previous
"""Trainium2 Bass kernel for nn_Attention (B=4, L=1024, D=768, H=12, DH=64).

Reference per (batch b, head n):
    A = q_n^T k_n                [D, D]
    scores = x A x^T             [L, L]
    S = softmax(scores, -1)
    V = v_n @ x_b^T              [DH, L]
    out[b, l, n*DH+e] = sum_m S[l, m] V[e, m] / sum_m S[l, m]

Sharding: 48 (b, n) units over 8 cores; core c owns batch pair c//4 and
head triple c%4.  A is computed once per head, reused for both batches;
heads 1,2 of each triple are split column-wise across the core pair and
exchanged with a 2-rank AllGather that overlaps head-0 compute.

fp8 acceleration: A is decomposed as abar*J + Atil (abar = mean, J =
ones).  q, k are uniform-positive so A is concentrated around its mean
-- quantizing it directly to e4m3 biases the dominant rank-1 logit term
abar*S(x)S(x)^T and blows the error budget.  Instead the zero-centered
Atil runs through fp8e4 DoubleRow matmuls (2 contraction chunks per
instruction, both operands fp8) for A, WT = Atil^T x^T and the scores,
while the exact rank-1 term is injected per scores-psum with a K=1
float32r matmul (lhsT = abar*2^15*S row, rhs = S row, host-computed in
f64).  The V/R path stays bf16 (V errors land directly in the output).

Scales (e4m3 max is 240 on TRN -- values above become Inf):
    q8, k8, v-host   x512
    A psum           = A_q8 * 2^18;  at = (psum - mean)*2^2 = Atil*2^20
    WT psum          = Atil*2^20 x;  wt = psum * 2^-5 = Wtil*2^15
    scores psum      = 2^15 * (x Atil x^T + abar S S^T);  exp scale 2^-15
"""

from contextlib import ExitStack

import ml_dtypes
import numpy as np

import concourse.tile as tile
from concourse import bacc, mybir
from concourse.bass import ts, ds
from concourse.bass_utils import run_bass_kernel_spmd

# If BASS_TRACE is set in an environment that lacks antenv.axon_hooks,
# run_bass_kernel_spmd's trace path would fail on import; register a
# fallback holder so tracing degrades gracefully instead.
try:
    import antenv.axon_hooks  # noqa: F401
except Exception:  # pragma: no cover
    import sys
    import types

    import antenv

    _m = types.ModuleType("antenv.axon_hooks")
    _m._hook = None
    _m.set_axon_ntff_profile_hook = lambda h: setattr(_m, "_hook", h)
    _m.get_axon_ntff_profile_hook = lambda: _m._hook
    sys.modules["antenv.axon_hooks"] = _m
    antenv.axon_hooks = _m

B, L, D, H = 4, 1024, 768, 12
DH = D // H          # 64
HPC = 3              # heads per core
BPC = 2              # batches per core
N_CORES = 8
DC = D // 128        # 6 chunks of the contraction/feature dim
JP = DC // 2         # 3 DoubleRow chunk-pairs
LB = L // 128        # 8 l-blocks / m-blocks
DHA = DH + 1         # 65: head slice width in vt_aug (ones column at 64)
DHALF = D // 2       # 384: A-half width for the pairwise exchange
F32 = mybir.dt.float32
F32R = mybir.dt.float32r
BF16 = mybir.dt.bfloat16
F8 = mybir.dt.float8e4
DR = mybir.MatmulPerfMode.DoubleRow
PAIR_GROUPS = [[0, 4], [1, 5], [2, 6], [3, 7]]

SQK = 512.0          # host scale on q, k
A_SC = 4.0           # at = (A_psum - mean) * 2^2  -> Atil * 2^20
W_SC = 2.0**-5       # wt = WT_psum * 2^-5         -> Wtil * 2^15
S_SC = 2.0**15       # scores psum scale; exp uses 1/S_SC
N_WARMUP = 28        # dummy matmuls to lift the HAM clock gate early

_COMPILED = None


def _build():
    nc = bacc.Bacc(
        "TRN2",
        target_bir_lowering=False,
        debug=False,
        enable_asserts=False,
        num_devices=N_CORES,
    )
    xT8_ext = nc.dram_tensor("xT8", [BPC, D, L], F8, kind="ExternalInput").ap()
    xTb_ext = nc.dram_tensor("xTb", [BPC, D, L], BF16, kind="ExternalInput").ap()
    q3_ext = nc.dram_tensor("q3", [HPC, D, D], F8, kind="ExternalInput").ap()
    k0_ext = nc.dram_tensor("k0", [D, D], F8, kind="ExternalInput").ap()
    kh_ext = nc.dram_tensor("kh", [2, D, DHALF], F8, kind="ExternalInput").ap()
    vT3_ext = nc.dram_tensor("vT3", [D, HPC * DH], BF16, kind="ExternalInput").ap()
    smh_ext = nc.dram_tensor("smh", [HPC, BPC, L], BF16, kind="ExternalInput").ap()
    smp_ext = nc.dram_tensor("smp", [BPC, L], BF16, kind="ExternalInput").ap()
    cneg_ext = nc.dram_tensor("cneg", [128, HPC], F32, kind="ExternalInput").ap()
    out_ext = nc.dram_tensor(
        "out_r", [BPC, L, HPC * DH], F32, kind="ExternalOutput"
    ).ap()

    with tile.TileContext(nc) as tc, ExitStack() as ctx:
        xt_pool = ctx.enter_context(tc.tile_pool(name="xt", bufs=1))
        vt3_pool = ctx.enter_context(tc.tile_pool(name="vt3", bufs=1))
        vt_pool = ctx.enter_context(tc.tile_pool(name="vt", bufs=1))
        qk_pool = ctx.enter_context(tc.tile_pool(name="qk", bufs=1))
        a_pool = ctx.enter_context(tc.tile_pool(name="a", bufs=1))
        wt_pool = ctx.enter_context(tc.tile_pool(name="wt", bufs=2))
        pt_pool = ctx.enter_context(tc.tile_pool(name="pt", bufs=2))
        soft_pool = ctx.enter_context(tc.tile_pool(name="soft", bufs=2))
        out_pool = ctx.enter_context(tc.tile_pool(name="outp", bufs=1))
        sm_pool = ctx.enter_context(tc.tile_pool(name="sm", bufs=1))
        dram_pool = ctx.enter_context(tc.tile_pool(name="dram", bufs=1, space="DRAM"))
        ps_p = ctx.enter_context(tc.tile_pool(name="ps_p", bufs=2, space="PSUM"))
        ps_s = ctx.enter_context(tc.tile_pool(name="ps_s", bufs=2, space="PSUM"))
        ps_r = ctx.enter_context(tc.tile_pool(name="ps_r", bufs=2, space="PSUM"))

        # ---------- HAM warmup: keep the PE busy from t=0 so the clock
        # gate lifts (~3.4us) before the real matmul stream begins.
        dummy = sm_pool.tile([128, 128], F8, tag="dummy")
        nc.gpsimd.memset(dummy[:], 0.0)
        for w in range(N_WARMUP):
            pw = ps_p.tile([128, 512], F32, tag=ps_p.name)
            nc.tensor.matmul(pw[:, :128], dummy[:], dummy[:], start=True, stop=True)

        # ---------- loads ----------
        def load3d(pool, tag, dram2d, width, dtype):
            t = pool.tile([128, DC, width], dtype, tag=tag)
            nc.sync.dma_start(t[:], dram2d.rearrange("(c p) w -> p c w", p=128))
            return t

        q_all = [None, None, None]
        q_all[0] = load3d(qk_pool, "q0", q3_ext[0], D, F8)
        k0_sb = load3d(qk_pool, "k0", k0_ext[:], D, F8)
        cneg_sb = sm_pool.tile([128, HPC], F32, tag="cneg")
        nc.sync.dma_start(cneg_sb[:], cneg_ext[:])
        smh_t = [[None] * BPC for _ in range(HPC)]
        smp_t = [None, None]
        for bi in range(BPC):
            t_smp = sm_pool.tile([1, L], BF16, tag=f"smp{bi}")
            nc.sync.dma_start(t_smp[:], smp_ext[ds(bi, 1), :])
            smp_t[bi] = t_smp
            for h in range(HPC):
                t_smh = sm_pool.tile([1, L], BF16, tag=f"smh{h}_{bi}")
                nc.sync.dma_start(t_smh[:], smh_ext[h, ds(bi, 1), :])
                smh_t[h][bi] = t_smh
        xt8 = [None, None]
        xtb = [None, None]
        xt8[0] = load3d(xt_pool, "x8_0", xT8_ext[0], L, F8)
        kh_sb = [None, None]
        for h in (1, 2):
            q_all[h] = load3d(qk_pool, f"q{h}", q3_ext[h], D, F8)
            kh_sb[h - 1] = load3d(qk_pool, f"kh{h}", kh_ext[h - 1], DHALF, F8)
        xtb[0] = load3d(xt_pool, "xb_0", xTb_ext[0], L, BF16)
        vt3 = load3d(vt3_pool, "vt3", vT3_ext[:], HPC * DH, BF16)
        xt8[1] = load3d(xt_pool, "x8_1", xT8_ext[1], L, F8)
        xtb[1] = load3d(xt_pool, "xb_1", xTb_ext[1], L, BF16)

        # at[h][p, j, d'] = Atil[128j+p, d'] * 2^20 in fp8
        at = [
            a_pool.tile([128, DC, D], F8, tag=f"at{h}", name=f"at{h}")
            for h in range(HPC)
        ]

        # ---- head 0: full Atil locally ----
        for i in range(DC):
            for n in range(2):
                pool = ps_p if n == 0 else ps_r
                p = pool.tile([128, 512], F32, tag=pool.name)
                for j in range(JP):
                    nc.tensor.matmul(
                        p[:, :DHALF],
                        q_all[0][:, 2 * j : 2 * j + 2, ts(i, 128)],
                        k0_sb[:, 2 * j : 2 * j + 2, ds(n * DHALF, DHALF)],
                        start=(j == 0),
                        stop=(j == JP - 1),
                        perf_mode=DR,
                    )
                nc.scalar.activation(
                    at[0][:, i, ds(n * DHALF, DHALF)],
                    p[:, :DHALF],
                    mybir.ActivationFunctionType.Identity,
                    bias=cneg_sb[:, 0:1],
                    scale=A_SC,
                )

        # ---- heads 1, 2: own half of Atil + background pairwise AllGather ----
        ah_dr = dram_pool.tile([2 * D, DHALF], F8, tag="ah_dr")
        gth_dr = dram_pool.tile([4 * D, DHALF], F8, tag="gth")
        for h in (1, 2):
            for i in range(DC):
                pool = ps_p if i % 2 == 0 else ps_r
                p = pool.tile([128, 512], F32, tag=pool.name)
                for j in range(JP):
                    nc.tensor.matmul(
                        p[:, :DHALF],
                        q_all[h][:, 2 * j : 2 * j + 2, ts(i, 128)],
                        kh_sb[h - 1][:, 2 * j : 2 * j + 2, :],
                        start=(j == 0),
                        stop=(j == JP - 1),
                        perf_mode=DR,
                    )
                ao = a_pool.tile([128, DHALF], F8, tag=f"ao{h}_{i}")
                nc.scalar.activation(
                    ao[:],
                    p[:, :DHALF],
                    mybir.ActivationFunctionType.Identity,
                    bias=cneg_sb[:, h : h + 1],
                    scale=A_SC,
                )
                nc.sync.dma_start(ah_dr[ds((h - 1) * D + 128 * i, 128), :], ao[:])
        nc.gpsimd.collective_compute(
            "AllGather",
            mybir.AluOpType.bypass,
            replica_groups=PAIR_GROUPS,
            ins=[ah_dr[:].opt()],
            outs=[gth_dr[:].opt()],
        )
        # gathered rows: [rank, h-1, d]; rank 0 owns cols 0:384, rank 1 the rest
        for h in (1, 2):
            for i in range(DC):
                nc.sync.dma_start(
                    at[h][:, i, ds(0, DHALF)],
                    gth_dr[ds((h - 1) * D + 128 * i, 128), :],
                )
                nc.sync.dma_start(
                    at[h][:, i, ds(DHALF, DHALF)],
                    gth_dr[ds(2 * D + (h - 1) * D + 128 * i, 128), :],
                )

        # ---------- VT_aug projection per batch (bf16, as V errors are
        # first-order in the output) ----------
        vt = [None, None]

        def build_vt(bi):
            tiles = []
            for j in range(LB):
                pool = ps_p if j % 2 == 0 else ps_r
                p = pool.tile([128, 512], F32, tag=pool.name)
                for i in range(DC):
                    nc.tensor.matmul(
                        p[:, : HPC * DH],
                        xtb[bi][:, i, ts(j, 128)],
                        vt3[:, i, :],
                        start=(i == 0),
                        stop=(i == DC - 1),
                    )
                t = vt_pool.tile([128, HPC * DHA], BF16, tag=f"vt{bi}_{j}")
                nc.gpsimd.memset(t[:], 1.0)
                t3 = t[:].rearrange("p (h c) -> p h c", h=HPC)
                p3 = p[:, : HPC * DH].rearrange("p (h c) -> p h c", h=HPC)
                nc.vector.tensor_copy(t3[:, :, :DH], p3[:])
                tiles.append(t)
            vt[bi] = tiles

        build_vt(0)

        # out accumulators: per batch, one [128, 192] f32 tile per l-block
        out_sb = [[], []]
        for bi in range(BPC):
            for lb in range(LB):
                ot = out_pool.tile([128, HPC * DH], F32, tag=f"out{bi}_{lb}")
                out_sb[bi].append(ot)

        def emit_r(pt_sb, bi, h):
            # R per l-block + fused sums -> normalize (+ final out DMA)
            for lb in range(LB):
                pr = ps_r.tile([128, DHA], F32, tag=ps_r.name)
                for mj in range(LB):
                    nc.tensor.matmul(
                        pr[:],
                        pt_sb[mj][:, ts(lb, 128)],
                        vt[bi][mj][:, ds(DHA * h, DHA)],
                        start=(mj == 0),
                        stop=(mj == LB - 1),
                    )
                recip = soft_pool.tile([128, 1], F32, tag="recip")
                nc.vector.reciprocal(recip[:], pr[:, DH : DH + 1])
                nc.vector.tensor_scalar_mul(
                    out_sb[bi][lb][:, ts(h, DH)], pr[:, :DH], recip[:]
                )
                if h == HPC - 1:
                    nc.sync.dma_start(
                        out_ext[bi, ts(lb, 128), :], out_sb[bi][lb][:]
                    )

        pending_r = None
        for h in range(HPC):
            for bi in range(BPC):
                # ---- WT[d', l] = sum_d Atil[d, d'] xT[d, l] * 2^20 ----
                wt = wt_pool.tile([128, DC, L], F8, tag="wt")
                for i in range(DC):
                    for n in range(2):
                        p = ps_p.tile([128, 512], F32, tag=ps_p.name)
                        for j in range(JP):
                            nc.tensor.matmul(
                                p[:],
                                at[h][:, 2 * j : 2 * j + 2, ts(i, 128)],
                                xt8[bi][:, 2 * j : 2 * j + 2, ts(n, 512)],
                                start=(j == 0),
                                stop=(j == JP - 1),
                                perf_mode=DR,
                            )
                        nc.vector.tensor_scalar_mul(
                            wt[:, i, ts(n, 512)], p[:], W_SC
                        )
                    if i == 0 and pending_r is not None:
                        emit_r(*pending_r)
                        pending_r = None

                if h == 0 and bi == 1:
                    build_vt(1)

                # ---- scoresT blocks: rank-1 preload + fp8 DR accumulate ----
                def scores_t(mj):
                    p = ps_s.tile([128, L], F32, tag="ps_s")
                    for n in range(2):
                        nc.tensor.matmul(
                            p[:, ts(n, 512)],
                            smh_t[h][bi][:, ts(mj, 128)],
                            smp_t[bi][:, ts(n, 512)],
                            start=True,
                            stop=False,
                            skip_group_check=True,
                        )
                        for j in range(JP):
                            nc.tensor.matmul(
                                p[:, ts(n, 512)],
                                xt8[bi][:, 2 * j : 2 * j + 2, ts(mj, 128)],
                                wt[:, 2 * j : 2 * j + 2, ts(n, 512)],
                                start=False,
                                stop=(j == JP - 1),
                                perf_mode=DR,
                                skip_group_check=True,
                            )
                    return p

                pt_sb = []
                ps_prev = scores_t(0)
                for mj in range(LB):
                    ps_cur = ps_prev
                    if mj + 1 < LB:
                        ps_prev = scores_t(mj + 1)
                    pt = pt_pool.tile([128, L], BF16, tag=f"pt{mj}")
                    nc.scalar.activation(
                        pt[:],
                        ps_cur[:],
                        mybir.ActivationFunctionType.Exp,
                        scale=1.0 / S_SC,
                    )
                    pt_sb.append(pt)
                pending_r = (pt_sb, bi, h)

        emit_r(*pending_r)

    nc.compile()
    return nc


def kernel(x, k, q, v):
    global _COMPILED
    if _COMPILED is None:
        _COMPILED = _build()

    x = np.ascontiguousarray(x, dtype=np.float32)
    k = np.ascontiguousarray(k, dtype=np.float32)
    q = np.ascontiguousarray(q, dtype=np.float32)
    v = np.ascontiguousarray(v, dtype=np.float32)

    bf = ml_dtypes.bfloat16
    f8 = ml_dtypes.float8_e4m3
    xT = x.transpose(0, 2, 1)              # [B, D, L]
    xT8 = xT.astype(f8)
    xTb = xT.astype(bf)
    q8 = (q * SQK).astype(f8)
    k8 = (k * SQK).astype(f8)
    vb = v.transpose(2, 0, 1).astype(bf)   # [D, H, DH]

    # exact rank-1 pieces (f64): abar = mean(q^T k), S = row sums of x
    q64 = q.astype(np.float64)
    k64 = k.astype(np.float64)
    abar = (q64.sum(axis=2) * k64.sum(axis=2)).sum(axis=1) / (D * D)  # [H]
    S = x.astype(np.float64).sum(axis=2)                              # [B, L]
    # mean of the *quantized* A (what the device must subtract)
    q8f = q8.astype(np.float64)
    k8f = k8.astype(np.float64)
    c_ps = (q8f.sum(axis=2) * k8f.sum(axis=2)).sum(axis=1) / (D * D)  # [H]
    smh = (abar[:, None, None] * S_SC * S[None]).astype(bf)           # [H, B, L]
    smp = S.astype(bf)                                                # [B, L]

    in_maps = []
    for c in range(N_CORES):
        bp, t = c // 4, c % 4
        hs = slice(HPC * t, HPC * (t + 1))
        h0 = HPC * t
        bsl = slice(BPC * bp, BPC * (bp + 1))
        cols = slice(DHALF * bp, DHALF * (bp + 1))
        in_maps.append(
            {
                "xT8": np.ascontiguousarray(xT8[bsl]),
                "xTb": np.ascontiguousarray(xTb[bsl]),
                "q3": np.ascontiguousarray(q8[hs]),
                "k0": np.ascontiguousarray(k8[h0]),
                "kh": np.ascontiguousarray(k8[h0 + 1 : h0 + 3, :, cols]),
                "vT3": np.ascontiguousarray(vb[:, hs].reshape(D, HPC * DH)),
                "smh": np.ascontiguousarray(smh[hs, bsl]),
                "smp": np.ascontiguousarray(smp[bsl]),
                "cneg": np.ascontiguousarray(
                    np.broadcast_to(
                        (-A_SC * c_ps[hs]).astype(np.float32)[None, :], (128, HPC)
                    )
                ),
            }
        )

    res = run_bass_kernel_spmd(_COMPILED, in_maps, core_ids=list(range(N_CORES)))

    out = np.empty((B, L, D), np.float32)
    for c in range(N_CORES):
        bp, t = c // 4, c % 4
        for bi in range(BPC):
            out[BPC * bp + bi, :, HPC * DH * t : HPC * DH * (t + 1)] = res.results[
                c
            ]["out_r"][bi]
    return out


if __name__ == "__main__":
    rng = np.random.default_rng(0)
    x = rng.standard_normal((B, L, D)).astype(np.float32)
    k = (rng.random((H, D, D)) / D).astype(np.float32)
    q = (rng.random((H, D, D)) / D).astype(np.float32)
    v = (rng.random((H, DH, D)) / D).astype(np.float32)
    o = kernel(x=x, k=k, q=q, v=v)
    print("out", o.shape, o.dtype)
